# revision 30
# baseline (speedup 1.0000x reference)
"""HEATNet4 Bass/Tile kernel for 8 TRN2 NeuronCores.

Self-contained: takes FULL unsharded inputs, shards internally
(nodes row-sharded; edges sharded by dst owner), runs one SPMD Bass
program on cores 0-7, gathers the full [NI, 512] output.

I/O packing: the per-call dispatch overhead of this runtime scales
with the NUMBER of input tensors (~1.8 ms each) plus bytes, so all
inputs are packed into 4 tensors per core:
  fblob  bf16 [512, 4375]   transposed node features (img|gene|text)
  wshard f32  [NW/8]        1/8 shard of all weights (AllGathered on
                            device into wfull at kernel start)
  eb16   i16  [N16]         edge index data (16-row wrapped gather
                            indices, replicated to 128 on device)
  ebf    f32  [NF]          per-edge float data (dst pos, sim)
Runtime scalars (e_w/e_b folds, sigmoid(skip) blends) travel in
wshard as 128-replicated words read as [P,1] columns.
"""
import sys
sys.path.insert(0, "/opt/trn_rl_repo")

import math

import numpy as np

import concourse.bass as bass
import concourse.bacc as bacc
import concourse.mybir as mybir
import concourse.tile as tile
import concourse.bass_isa as bass_isa
from concourse.bass_utils import run_bass_kernel_spmd

F32 = mybir.dt.float32
BF16 = mybir.dt.bfloat16
I32 = mybir.dt.int32
I16 = mybir.dt.int16
NPBF16 = mybir.dt.np(BF16)
NCORES = 8
P = 128


class Cfg:
    NI, NG, NT = 20000, 10000, 5000
    D_IN, D, L, H = 512, 256, 2, 8
    DK = D // H
    E = 100000
    NS = [NI, NG, NT]
    OWN = [n // NCORES for n in NS]
    OWN_ALL = sum(OWN)
    TBASE = [0, OWN[0], OWN[0] + OWN[1]]


ETYPES = [  # (st, dt, suffix)
    (0, 1, "ig"), (1, 0, "gi"), (0, 2, "it"),
    (2, 0, "ti"), (1, 2, "gt"), (2, 1, "tg"),
]


# ---------------------------------------------------------------- host prep

def _wrap16(idx: np.ndarray) -> np.ndarray:
    """[n] -> [16, n/16] wrapped int16 layout for dma_gather idx."""
    n = idx.shape[0]
    return np.ascontiguousarray(idx.astype(np.int16).reshape(n // 16, 16).T)


def _bin_pack(deg: np.ndarray, B: int):
    """LPT: assign nodes to B bins (<=128 nodes each), balancing edge load."""
    import heapq
    order = np.argsort(-deg, kind="stable")
    bins = [[] for _ in range(B)]
    heap = [(0, b) for b in range(B)]
    heapq.heapify(heap)
    for n in order:
        while True:
            load, b = heapq.heappop(heap)
            if len(bins[b]) < P:
                break
        bins[b].append(int(n))
        heapq.heappush(heap, (load + int(deg[n]), b))
    return bins


class _Pk:
    """Packs named f32 arrays into one flat buffer, recording offsets."""

    def __init__(self):
        self.parts, self.offs, self.off = [], {}, 0

    def add(self, name, arr):
        a = np.ascontiguousarray(np.asarray(arr), ).astype(np.float32).ravel()
        self.offs[name] = self.off
        self.parts.append(a)
        self.off += a.size

    def finish(self, pad_mult):
        tot = self.off
        padded = ((tot + pad_mult - 1) // pad_mult) * pad_mult
        if padded > tot:
            self.parts.append(np.zeros(padded - tot, np.float32))
        return np.concatenate(self.parts), padded


def prep(inputs: dict, cfg: Cfg):
    """Host-side preprocessing. Returns (in_maps, struct)."""
    OWN, TBASE = cfg.OWN, cfg.TBASE
    e_w, e_b = np.asarray(inputs["e_w"]), np.asarray(inputs["e_b"])
    inv_sqrt_dk = 1.0 / math.sqrt(cfg.DK)

    struct = {}
    per_core_16 = [[] for _ in range(NCORES)]   # list of i16 flats
    per_core_f = [[] for _ in range(NCORES)]    # list of f32 flats
    o16 = of = 0

    for st, dt, sfx in ETYPES:
        src = np.asarray(inputs[f"src_{sfx}"]).astype(np.int64)
        dst = np.asarray(inputs[f"dst_{sfx}"]).astype(np.int64)
        sim = np.asarray(inputs[f"sim_{sfx}"]).astype(np.float32)
        own = OWN[dt]
        owner = dst // own

        cores = []
        for c in range(NCORES):
            eids = np.nonzero(owner == c)[0]
            dl = dst[eids] - c * own
            order = np.argsort(dl, kind="stable")
            eids = eids[order]
            dl = dl[order]
            deg = np.bincount(dl, minlength=own)
            starts = np.zeros(own + 1, np.int64)
            np.cumsum(deg, out=starts[1:])
            cores.append((eids, deg, starts))

        B0 = (own + P - 1) // P
        best = None
        for B in range(B0, B0 + 4):
            allbins = []
            C = 1
            for c in range(NCORES):
                bins = _bin_pack(cores[c][1], B)
                allbins.append(bins)
                for bn in bins:
                    load = int(cores[c][1][bn].sum()) if bn else 0
                    C = max(C, (load + P - 1) // P)
            tot = B * C
            if best is None or tot < best[0]:
                best = (tot, B, C, allbins)
        _, B, C, allbins = best
        nch = B * C
        epad = nch * P

        struct[sfx] = dict(B=B, C=C, nch=nch,
                           o_srcg=o16, o_qg=o16 + 16 * nch * 8,
                           o_sidx=o16 + 32 * nch * 8,
                           o_dcol=of, o_sim=of + P * nch)
        o16 += 32 * nch * 8 + P * B
        of += 2 * P * nch

        for c in range(NCORES):
            eids, deg, starts = cores[c]
            src_arr = np.zeros(epad, np.int64)
            q_arr = np.zeros(epad, np.int64)
            pos_arr = np.full(epad, -1.0, np.float32)
            sim_arr = np.zeros(epad, np.float32)
            sidx = np.full((P, B), own, np.int64)
            for b, bn in enumerate(allbins[c]):
                cur = b * C * P
                for pos, n in enumerate(bn):
                    sidx[pos, b] = n
                    s0, s1 = starts[n], starts[n + 1]
                    k = s1 - s0
                    if k:
                        sel = eids[s0:s1]
                        src_arr[cur:cur + k] = src[sel]
                        q_arr[cur:cur + k] = TBASE[dt] + (dst[sel] - c * own)
                        pos_arr[cur:cur + k] = pos
                        sim_arr[cur:cur + k] = sim[sel]
                        cur += k
                assert cur <= (b + 1) * C * P

            per_core_16[c] += [_wrap16(src_arr).ravel(),
                               _wrap16(q_arr).ravel(),
                               sidx.astype(np.int16).ravel()]
            per_core_f[c] += [pos_arr.reshape(nch, P).T.ravel(),
                              sim_arr.reshape(nch, P).T.ravel()]

    # ---- weights blob (shared across cores; shipped sharded):
    # f32 part (biases, runtime scalars, iota/ident) followed by a bf16
    # section holding all weight matrices (cast back to f32 on device)
    pk = _Pk()
    pk.add("adapt_b", inputs["adapt_b"])
    pk.add("k_b", inputs["k_b"])
    pk.add("q_b", inputs["q_b"])
    pk.add("v_b", inputs["v_b"])
    pk.add("a_b", inputs["a_b"])
    pk.add("pred_b", np.asarray(inputs["pred_b"], np.float32)[0])
    pk.add("head1_b", inputs["head1_b"])
    pk.add("head_b", inputs["head_b"])
    s_ew = (e_w * inv_sqrt_dk).astype(np.float32)      # [L]
    s_eb = (e_b * inv_sqrt_dk).astype(np.float32)
    pk.add("s_ew", np.repeat(s_ew, P))
    pk.add("s_eb", np.repeat(s_eb, P))
    alpha = 1.0 / (1.0 + np.exp(-np.asarray(inputs["skip"], np.float64)))
    pk.add("alpha", np.repeat(alpha.astype(np.float32).ravel(), P))
    pk.add("oma", np.repeat((1.0 - alpha).astype(np.float32).ravel(), P))
    pk.add("iota", np.tile(np.arange(P, dtype=np.float32), (P, 1)))
    pk.add("ident", np.eye(P, dtype=np.float32))
    wpart, WMAT = pk.finish(pad_mult=2)
    offs16, m16, o16w = {}, [], 0
    for name, arr in (
            ("adapt_w", inputs["adapt_w"]),
            ("k_w", inputs["k_w"]), ("q_w", inputs["q_w"]),
            ("v_w", inputs["v_w"]),
            ("a_w", np.asarray(inputs["a_w"], np.float32) * 0.5),
            ("pred_w", np.asarray(inputs["pred_w"], np.float32)[0] / cfg.NI),
            ("head1_w", inputs["head1_w"]),
            ("head_w", inputs["head_w"])):
        a = np.ascontiguousarray(np.asarray(arr)).astype(NPBF16).ravel()
        offs16[name] = o16w
        m16.append(a)
        o16w += a.size
    if o16w % 2:
        m16.append(np.zeros(1, NPBF16)); o16w += 1
    wblob = np.concatenate(
        [wpart.view(np.uint8), np.concatenate(m16).view(np.uint8)]
    ).view(np.float32)
    NW = WMAT + o16w // 2
    NW = ((NW + NCORES * 4 - 1) // (NCORES * 4)) * (NCORES * 4)
    SW = NW // NCORES
    wblob = np.concatenate([wblob, np.zeros(NW - wblob.size, np.float32)])

    # single-blob layout (f32 words): [wshard | fblob(bf16) | eb16(i16) | ebf]
    FCOLS = cfg.OWN_ALL + (-cfg.OWN_ALL) % 2
    N16 = o16
    assert N16 % 2 == 0
    O_F = SW
    O_16 = O_F + cfg.D_IN * FCOLS // 2
    O_EF = O_16 + N16 // 2
    NB = O_EF + of
    NB += (-NB) % 128
    struct["pack"] = dict(offs=pk.offs, offs16=offs16, WMAT=WMAT,
                          NW=NW, SW=SW, N16=N16, NF=of,
                          FCOLS=FCOLS, O_F=O_F, O_16=O_16, O_EF=O_EF, NB=NB)

    feats = [np.asarray(inputs["feat_image"], np.float32),
             np.asarray(inputs["feat_gene"], np.float32),
             np.asarray(inputs["feat_text"], np.float32)]
    in_maps = []
    for c in range(NCORES):
        fb = np.zeros((cfg.D_IN, FCOLS), NPBF16)
        fb[:, :cfg.OWN_ALL] = np.concatenate(
            [feats[t][c * cfg.OWN[t]:(c + 1) * cfg.OWN[t]].T
             for t in range(3)], axis=1).astype(NPBF16)
        blob = np.concatenate([
            wblob[c * SW:(c + 1) * SW].view(np.uint8),
            np.ascontiguousarray(fb).view(np.uint8).ravel(),
            np.concatenate(per_core_16[c]).view(np.uint8),
            np.concatenate(per_core_f[c]).view(np.uint8),
        ]).view(np.float32)
        blob = np.concatenate([blob, np.zeros(NB - blob.size, np.float32)])
        in_maps.append(dict(blob=blob.reshape(NB // 128, 128)))
    return in_maps, struct


# ---------------------------------------------------------------- device build

BUILD_MODE = "full"  # "full" | "nogather" (timing expt) | "stub" (dispatch baseline)


def build(struct, cfg: Cfg):
    OWN, TBASE, NS = cfg.OWN, cfg.TBASE, cfg.NS
    D, L, H, DK, D_IN = cfg.D, cfg.L, cfg.H, cfg.DK, cfg.D_IN
    OWN_ALL = cfg.OWN_ALL
    KI_IN, MO = D_IN // P, D // P  # 4, 2
    KI = D // P                    # 2
    PACK = struct["pack"]
    OFFS, NW, SW = PACK["offs"], PACK["NW"], PACK["SW"]

    nc = bacc.Bacc("TRN2", target_bir_lowering=False, debug=False,
                   num_devices=NCORES)

    # NOTE: declared 2-D — the transfer layer ships large 1-D tensors ~2x
    # slower than the same bytes declared [N/128, 128].
    blob2 = nc.dram_tensor("blob", [PACK["NB"] // 128, 128], F32,
                           kind="ExternalInput")
    blob = blob2.rearrange("r c -> (r c)")
    FCOLS, O_F, O_16, O_EF = (PACK["FCOLS"], PACK["O_F"], PACK["O_16"],
                              PACK["O_EF"])
    wshard = blob[0:SW]
    fblob = blob[O_F:O_F + D_IN * FCOLS // 2].bitcast(BF16).rearrange(
        "(r c) -> r c", c=FCOLS)

    def eb16v(o, sz, cols):
        assert o % 2 == 0 and sz % 2 == 0
        return blob[O_16 + o // 2:O_16 + (o + sz) // 2].bitcast(
            I16).rearrange("(r c) -> r c", c=cols)

    def ebfv(o, sz, cols):
        return blob[O_EF + o:O_EF + o + sz].rearrange("(r c) -> r c", c=cols)

    out = nc.dram_tensor("out", [OWN[0], D_IN], BF16, kind="ExternalOutput")

    # ---- internal DRAM
    wsh_int = nc.dram_tensor("wsh_int", [SW], F32)
    wfull = nc.dram_tensor("wfull", [NW], F32, addr_space="Shared")
    h_cur = [nc.dram_tensor(f"hA_{t}", [D, OWN[t]], F32) for t in range(3)]
    h_nxt = [nc.dram_tensor(f"hB_{t}", [D, OWN[t]], F32) for t in range(3)]
    kv_own = [nc.dram_tensor(f"kvown_{t}", [OWN[t], 2 * D], BF16)
              for t in range(3)]
    kv_full = [nc.dram_tensor(f"kvfull_{t}", [NS[t], 2 * D], BF16,
                              addr_space="Shared") for t in range(3)]
    q_loc = nc.dram_tensor("q_loc", [OWN_ALL, D], BF16)
    agg_t = {sfx: nc.dram_tensor(f"agg_{sfx}", [OWN[dt_] + 1, D], F32)
             for st, dt_, sfx in ETYPES}
    pool_in = nc.dram_tensor("pool_in", [D, 1], F32)
    pool_ar = nc.dram_tensor("pool_ar", [D, 1], F32, addr_space="Shared")

    RG = [list(range(NCORES))]

    def wv(name, rows, cols, extra_off=0):
        """2-D view of an f32 entry stored row-major in wfull."""
        o = OFFS[name] + extra_off
        return wfull[o:o + rows * cols].rearrange("(r c) -> r c", c=cols)

    def wv16(name, rows, cols, extra_off=0):
        """2-D view of a bf16 weight matrix in wfull's bf16 section."""
        o = PACK["offs16"][name] + extra_off
        base = PACK["WMAT"]
        return wfull[base + o // 2:base + (o + rows * cols) // 2].bitcast(
            BF16).rearrange("(r c) -> r c", c=cols)

    with tile.TileContext(nc) as tc:
        with (
            tc.tile_pool(name="cst", bufs=1) as cst,
            tc.tile_pool(name="wts", bufs=1) as wts,
            tc.tile_pool(name="act", bufs=2) as act,
            tc.tile_pool(name="gath", bufs=2) as gath,
            tc.tile_pool(name="etc", bufs=2) as etc_p,
            tc.tile_pool(name="sml", bufs=4) as sml,
            tc.tile_pool(name="ps", bufs=2, space="PSUM") as ps,
        ):
            # ---- gather full weight blob
            nc.sync.dma_start(out=wsh_int[:], in_=wshard[:])
            nc.gpsimd.collective_compute(
                "AllGather", mybir.AluOpType.bypass, replica_groups=RG,
                ins=[wsh_int[:]], outs=[wfull[:]])

            iota = cst.tile([P, P], F32)
            nc.sync.dma_start(out=iota[:], in_=wv("iota", P, P))
            ident = cst.tile([P, P], F32)
            nc.sync.dma_start(out=ident[:], in_=wv("ident", P, P))
            iota_w = cst.tile([P, 9 * P], F32, tag="iota_w")
            for g in range(9):
                nc.sync.dma_start(out=iota_w[:, g * P:(g + 1) * P],
                                  in_=wv("iota", P, P))
            # runtime scalar columns: [P, 1] views
            scols = cst.tile([P, 2 * L + 4 * L * 3], F32, tag="scols")
            nc.sync.dma_start(
                out=scols[:, 0:2 * L],
                in_=wfull[OFFS["s_ew"]:OFFS["s_ew"] + 2 * L * P].rearrange(
                    "(m p) -> p m", p=P))
            nc.sync.dma_start(
                out=scols[:, 2 * L:2 * L + 2 * L * 3],
                in_=wfull[OFFS["alpha"]:OFFS["alpha"] + 2 * L * 3 * P].rearrange(
                    "(m p) -> p m", p=P))
            s_ew_c = [scols[:, l:l + 1] for l in range(L)]
            s_eb_c = [scols[:, L + l:L + l + 1] for l in range(L)]
            al_c = [[scols[:, 2 * L + l * 3 + t:2 * L + l * 3 + t + 1]
                     for t in range(3)] for l in range(L)]
            oma_c = [[scols[:, 5 * L + l * 3 + t:5 * L + l * 3 + t + 1]
                      for t in range(3)] for l in range(L)]

            def load_w_tiles(name, n_ki, n_mo, tag, extra_off=0):
                w_ap = wv16(name, n_ki * P, n_mo * P, extra_off)
                wide16 = wts.tile([P, n_ki * n_mo * P], BF16, tag=tag + "16")
                for ki in range(n_ki):
                    for mo in range(n_mo):
                        j = (ki * n_mo + mo) * P
                        nc.sync.dma_start(
                            out=wide16[:, j:j + P],
                            in_=w_ap[ki * P:(ki + 1) * P, mo * P:(mo + 1) * P])
                wide = wts.tile([P, n_ki * n_mo * P], F32, tag=tag)
                nc.vector.tensor_copy(out=wide[:], in_=wide16[:])
                return [[wide[:, (ki * n_mo + mo) * P:(ki * n_mo + mo + 1) * P]
                         for mo in range(n_mo)] for ki in range(n_ki)]

            def bias_cols(name, n_mo, tag, extra_off=0):
                o = OFFS[name] + extra_off
                wide = sml.tile([P, n_mo], F32, tag=tag)
                nc.sync.dma_start(
                    out=wide[:],
                    in_=wfull[o:o + n_mo * P].rearrange("(m p) -> p m", p=P))
                return [wide[:, mo:mo + 1] for mo in range(n_mo)]

            def rhs_wide(n_ki, tag, pool=None):
                wide = (pool or act).tile([P, n_ki * 512], F32, tag=tag)
                return wide, [wide[:, ki * 512:(ki + 1) * 512]
                              for ki in range(n_ki)]

            def load_feat(dest_ap, ki, col0, w):
                """DMA bf16 feature slice + cast to f32 into dest_ap[:, :w]."""
                bfst = gath.tile([P, 512], BF16, tag="bfst")
                nc.sync.dma_start(
                    out=bfst[:, :w],
                    in_=fblob[ki * P:(ki + 1) * P, col0:col0 + w])
                nc.vector.tensor_copy(out=dest_ap[:, :w], in_=bfst[:, :w])

            def linear_ft(w_tiles, b_cols, rhs_tiles, w, out_tag, alloc=512):
                n_ki = len(w_tiles)
                n_mo = len(w_tiles[0])
                ow = act.tile([P, n_mo * alloc], F32, tag=out_tag)
                outs = []
                for mo in range(n_mo):
                    psum = ps.tile([P, w], F32, tag="lin")
                    for ki in range(n_ki):
                        nc.tensor.matmul(out=psum[:], lhsT=w_tiles[ki][mo],
                                         rhs=rhs_tiles[ki][:, :w],
                                         start=(ki == 0), stop=(ki == n_ki - 1))
                    o = ow[:, mo * alloc:mo * alloc + w]
                    nc.vector.tensor_scalar_add(o, psum[:], b_cols[mo])
                    outs.append(ow[:, mo * alloc:(mo + 1) * alloc])
                return outs

            # ---------------- phase 0: adapt
            for t in range(3):
                w_tiles = load_w_tiles("adapt_w", KI_IN, MO, "adw",
                                       extra_off=t * D_IN * D)
                b_cols = bias_cols("adapt_b", MO, "adb", extra_off=t * D)
                own = OWN[t]
                for c0 in range(0, own, 512):
                    w = min(512, own - c0)
                    _, rhs = rhs_wide(KI_IN, "gf")
                    for ki in range(KI_IN):
                        load_feat(rhs[ki], ki, TBASE[t] + c0, w)
                    houts = linear_ft(w_tiles, b_cols, rhs, w, "hout")
                    for mo in range(MO):
                        nc.sync.dma_start(
                            out=h_cur[t][mo * P:(mo + 1) * P, c0:c0 + w],
                            in_=houts[mo][:, :w])

            # ---------------- layers
            for l in range(L if BUILD_MODE != "stub" else 0):
                # phase 1: k/q/v linears + transpose to row tables
                for t in range(3):
                    own = OWN[t]
                    eo = (l * 3 + t) * D * D
                    eob = (l * 3 + t) * D
                    kw_t = load_w_tiles("k_w", KI, MO, "kw", eo)
                    qw_t = load_w_tiles("q_w", KI, MO, "qw", eo)
                    vw_t = load_w_tiles("v_w", KI, MO, "vw", eo)
                    kb = bias_cols("k_b", MO, "kb", eob)
                    qb = bias_cols("q_b", MO, "qb", eob)
                    vb = bias_cols("v_b", MO, "vb", eob)
                    for c0 in range(0, own, 512):
                        w = min(512, own - c0)
                        _, rhs = rhs_wide(KI, "hrhs")
                        for ki in range(KI):
                            nc.sync.dma_start(
                                out=rhs[ki][:, :w],
                                in_=h_cur[t][ki * P:(ki + 1) * P, c0:c0 + w])
                        kT = linear_ft(kw_t, kb, rhs, w, "kT")
                        qT = linear_ft(qw_t, qb, rhs, w, "qT")
                        vT = linear_ft(vw_t, vb, rhs, w, "vT")
                        for s0 in range(0, w, P):
                            sw = min(P, w - s0)
                            kvrow = act.tile([P, 2 * D], BF16, tag="kvrow")
                            qrow = act.tile([P, D], BF16, tag="qrow")
                            for mo in range(MO):
                                for src_t, dst_col, buf in (
                                        (kT[mo], mo * P, kvrow),
                                        (vT[mo], D + mo * P, kvrow),
                                        (qT[mo], mo * P, qrow)):
                                    pt = ps.tile([P, P], F32, tag="tp")
                                    nc.tensor.transpose(
                                        out=pt[:sw, :],
                                        in_=src_t[:, s0:s0 + sw],
                                        identity=ident[:])
                                    nc.vector.tensor_copy(
                                        out=buf[:sw, dst_col:dst_col + P],
                                        in_=pt[:sw, :])
                            r0 = c0 + s0
                            nc.sync.dma_start(
                                out=kv_own[t][r0:r0 + sw, :],
                                in_=kvrow[:sw, :])
                            nc.sync.dma_start(
                                out=q_loc[TBASE[t] + r0:TBASE[t] + r0 + sw, :],
                                in_=qrow[:sw, :])
                    # allgather this type's kv table right away so the
                    # collective overlaps the next type's phase-1 compute
                    if BUILD_MODE == "nocoll":
                        nc.sync.dma_start(out=kv_full[t][0:OWN[t], :],
                                          in_=kv_own[t][:])
                    else:
                        nc.gpsimd.collective_compute(
                            "AllGather", mybir.AluOpType.bypass,
                            replica_groups=RG,
                            ins=[kv_own[t][:]], outs=[kv_full[t][:]])

                # phase 3: message passing per etype
                for st, dt_, sfx in ETYPES:
                    S_ = struct[sfx]; B, C, nch = S_["B"], S_["C"], S_["nch"]
                    n16 = nch * 8
                    srcg = etc_p.tile([P, n16], I16, tag="srcg")
                    qgi = etc_p.tile([P, n16], I16, tag="qgi")
                    for name, tl in (("o_srcg", srcg), ("o_qg", qgi)):
                        v16 = eb16v(S_[name], 16 * n16, n16)
                        for k in range(8):
                            nc.sync.dma_start(
                                out=tl[16 * k:16 * (k + 1), :], in_=v16)
                    dcol = etc_p.tile([P, nch], F32, tag="dcol")
                    nc.sync.dma_start(
                        out=dcol[:], in_=ebfv(S_["o_dcol"], P * nch, nch))
                    simt = etc_p.tile([P, nch], F32, tag="simt")
                    nc.sync.dma_start(
                        out=simt[:], in_=ebfv(S_["o_sim"], P * nch, nch))
                    ea = etc_p.tile([P, nch], F32, tag="ea")
                    nc.vector.tensor_scalar(
                        out=ea[:], in0=simt[:], scalar1=s_ew_c[l],
                        scalar2=s_eb_c[l], op0=mybir.AluOpType.mult,
                        op1=mybir.AluOpType.add)
                    sidx16 = etc_p.tile([P, B], I16, tag="sidx16")
                    nc.sync.dma_start(
                        out=sidx16[:], in_=eb16v(S_["o_sidx"], P * B, B))
                    sidx = etc_p.tile([P, B], I32, tag="sidx")
                    nc.vector.tensor_copy(out=sidx[:], in_=sidx16[:])

                    # single-pass message passing: gather full kv
                    # rows (k|v contiguous -> one gpsimd call) + q rows in
                    # waves; batch score/mask/weight math per wave; two PE
                    # matmuls per chunk accumulate numerator+denominator per
                    # dst block. Scores are O(5) so exp needs no max-shift.
                    GW = 9        # chunks per gather wave (desc ring <=2048)
                    psum = None
                    for k0 in range(0, nch, GW):
                        gc = min(GW, nch - k0)
                        kvg = gath.tile([P, GW, 2 * D], BF16, tag="kvg")
                        qg = gath.tile([P, GW, D], BF16, tag="qg")
                        if BUILD_MODE == "nogather":
                            nc.sync.dma_start(
                                out=kvg[:, :gc, :],
                                in_=kv_full[st][0:gc * P, :].rearrange(
                                    "(g p) d -> p g d", p=P))
                            nc.sync.dma_start(
                                out=qg[:, :gc, :],
                                in_=q_loc[0:gc * P, :].rearrange(
                                    "(g p) d -> p g d", p=P))
                        else:
                            nc.gpsimd.dma_gather(
                                kvg[:, :gc, :], kv_full[st][:],
                                srcg[:, k0 * 8:(k0 + gc) * 8],
                                gc * P, gc * P, 2 * D)
                            nc.gpsimd.dma_gather(
                                qg[:, :gc, :], q_loc[:],
                                qgi[:, k0 * 8:(k0 + gc) * 8],
                                gc * P, gc * P, D)
                        prod = gath.tile([P, GW, D], F32, tag="prod")
                        nc.vector.tensor_mul(
                            prod[:, :gc, :], kvg[:, :gc, 0:D], qg[:, :gc, :])
                        scw = gath.tile([P, GW, H], F32, tag="scw")
                        nc.vector.tensor_reduce(
                            out=scw[:, :gc, :],
                            in_=prod[:, :gc, :].rearrange(
                                "p g (h k) -> p g h k", k=DK),
                            axis=mybir.AxisListType.X, op=mybir.AluOpType.add)
                        nc.vector.tensor_mul(
                            scw[:, :gc, :], scw[:, :gc, :],
                            ea[:, k0:k0 + gc].to_broadcast([P, gc, H]))
                        nc.scalar.activation(
                            out=scw[:, :gc, :].rearrange("p g h -> p (g h)"),
                            in_=scw[:, :gc, :].rearrange("p g h -> p (g h)"),
                            func=mybir.ActivationFunctionType.Exp)
                        smw = gath.tile([P, GW, P], F32, tag="smw")
                        nc.vector.tensor_tensor(
                            out=smw[:, :gc, :],
                            in0=dcol[:, k0:k0 + gc].to_broadcast([P, gc, P]),
                            in1=iota_w[:, 0:gc * P].rearrange(
                                "p (g x) -> p g x", x=P),
                            op=mybir.AluOpType.is_equal)
                        vprod = gath.tile([P, GW, D], F32, tag="vprod")
                        nc.vector.tensor_mul(
                            vprod[:, :gc, :].rearrange(
                                "p g (h k) -> p g h k", k=DK),
                            kvg[:, :gc, D:2 * D].rearrange(
                                "p g (h k) -> p g h k", k=DK),
                            scw[:, :gc, :].to_broadcast([P, gc, H, DK]))
                        for j in range(gc):
                            k = k0 + j
                            ci = k % C
                            b = k // C
                            if ci == 0:
                                psum = ps.tile([P, H + D], F32, tag="blk")
                            rhs = etc_p.tile([P, H + D], F32, tag="rhs")
                            nc.vector.tensor_copy(out=rhs[:, 0:H],
                                                  in_=scw[:, j, :])
                            nc.vector.tensor_copy(out=rhs[:, H:H + D],
                                                  in_=vprod[:, j, :])
                            nc.tensor.matmul(
                                out=psum[:], lhsT=smw[:, j, :], rhs=rhs[:],
                                start=(ci == 0), stop=(ci == C - 1))
                            if ci == C - 1:
                                s_t = sml.tile([P, H], F32, tag="s_t")
                                nc.vector.tensor_scalar_max(
                                    s_t[:], psum[:, 0:H], 1e-30)
                                r_t = sml.tile([P, H], F32, tag="r_t")
                                nc.vector.reciprocal(r_t[:], s_t[:])
                                aggsc = etc_p.tile([P, D], F32, tag="aggsc")
                                nc.vector.tensor_mul(
                                    aggsc[:].rearrange(
                                        "p (h k) -> p h k", k=DK),
                                    psum[:, H:H + D].rearrange(
                                        "p (h k) -> p h k", k=DK),
                                    r_t[:].to_broadcast([P, H, DK]))
                                nc.gpsimd.indirect_dma_start(
                                    out=agg_t[sfx][:],
                                    out_offset=bass.IndirectOffsetOnAxis(
                                        ap=sidx[:, b:b + 1], axis=0),
                                    in_=aggsc[:], in_offset=None)

                # phase 4: combine aggs, a_w transform, blend
                for t in range(3):
                    own = OWN[t]
                    sfxs = [sfx for st, dt_, sfx in ETYPES if dt_ == t]
                    eo = (l * 3 + t) * D * D
                    eob = (l * 3 + t) * D
                    aw_t = load_w_tiles("a_w", KI, MO, "aw", eo)
                    ab = bias_cols("a_b", MO, "ab", eob)
                    for r0 in range(0, own, P):
                        rw = min(P, own - r0)
                        asum = act.tile([P, D], F32, tag="asum")
                        a2 = act.tile([P, D], F32, tag="a2")
                        nc.sync.dma_start(
                            out=asum[:rw, :],
                            in_=agg_t[sfxs[0]][r0:r0 + rw, :])
                        nc.sync.dma_start(
                            out=a2[:rw, :], in_=agg_t[sfxs[1]][r0:r0 + rw, :])
                        nc.vector.tensor_add(
                            asum[:rw, :], asum[:rw, :], a2[:rw, :])
                        aT_w = act.tile([P, KI * P], F32, tag="aT")
                        aT = []
                        for ki in range(KI):
                            pt = ps.tile([P, P], F32, tag="tp")
                            nc.tensor.transpose(
                                out=pt[:, :rw],
                                in_=asum[:rw, ki * P:(ki + 1) * P],
                                identity=ident[:rw, :rw])
                            a_sb = aT_w[:, ki * P:(ki + 1) * P]
                            nc.vector.tensor_copy(a_sb[:, :rw], pt[:, :rw])
                            aT.append(a_sb)
                        for mo in range(MO):
                            psum = ps.tile([P, P], F32, tag="lin")
                            for ki in range(KI):
                                nc.tensor.matmul(
                                    out=psum[:, :rw], lhsT=aw_t[ki][mo][:],
                                    rhs=aT[ki][:, :rw],
                                    start=(ki == 0), stop=(ki == KI - 1))
                            hold = act.tile([P, P], F32, tag="hold")
                            nc.sync.dma_start(
                                out=hold[:, :rw],
                                in_=h_cur[t][mo * P:(mo + 1) * P, r0:r0 + rw])
                            # new = (psum + ab) * al + hold * (1 - al)
                            tr = act.tile([P, P], F32, tag="tr")
                            nc.vector.tensor_scalar(
                                out=tr[:, :rw], in0=psum[:, :rw],
                                scalar1=ab[mo], scalar2=al_c[l][t],
                                op0=mybir.AluOpType.add,
                                op1=mybir.AluOpType.mult)
                            nc.vector.tensor_scalar_mul(
                                hold[:, :rw], hold[:, :rw], oma_c[l][t])
                            nc.vector.tensor_add(
                                tr[:, :rw], tr[:, :rw], hold[:, :rw])
                            nc.sync.dma_start(
                                out=h_nxt[t][mo * P:(mo + 1) * P, r0:r0 + rw],
                                in_=tr[:, :rw])
                h_cur, h_nxt = h_nxt, h_cur

            # ---------------- phase 5: pool image + head
            for ki in range(KI if BUILD_MODE != "stub" else 0):
                pcol = sml.tile([P, 1], F32, tag="pcol")
                psub = sml.tile([P, 1], F32, tag="psub")
                for i, c0 in enumerate(range(0, OWN[0], 512)):
                    w = min(512, OWN[0] - c0)
                    htile = act.tile([P, 512], F32, tag="hrhs")
                    nc.sync.dma_start(
                        out=htile[:, :w],
                        in_=h_cur[0][ki * P:(ki + 1) * P, c0:c0 + w])
                    tgt = pcol if i == 0 else psub
                    nc.vector.tensor_reduce(
                        out=tgt[:], in_=htile[:, :w],
                        axis=mybir.AxisListType.X, op=mybir.AluOpType.add)
                    if i > 0:
                        nc.vector.tensor_add(pcol[:], pcol[:], psub[:])
                nc.sync.dma_start(
                    out=pool_in[ki * P:(ki + 1) * P, :], in_=pcol[:])
            if BUILD_MODE == "nocoll":
                nc.sync.dma_start(out=pool_ar[:], in_=pool_in[:])
            elif BUILD_MODE != "stub":
                nc.gpsimd.collective_compute(
                    "AllReduce", mybir.AluOpType.add, replica_groups=RG,
                    ins=[pool_in[:]], outs=[pool_ar[:]])
            pooled_w = sml.tile([P, KI], F32, tag="pooled")
            pooled = []
            for ki in range(KI):
                if BUILD_MODE == "stub":
                    nc.gpsimd.memset(pooled_w[:, ki:ki + 1], 0.0)
                else:
                    nc.sync.dma_start(
                        out=pooled_w[:, ki:ki + 1],
                        in_=pool_ar[ki * P:(ki + 1) * P, :])
                pooled.append(pooled_w[:, ki:ki + 1])
            pw_t = load_w_tiles("pred_w", KI, MO, "pw")
            pb = bias_cols("pred_b", MO, "pb")
            out0 = linear_ft(pw_t, pb, pooled, 1, "out0", alloc=1)
            h1_t = load_w_tiles("head1_w", KI, KI_IN, "h1w")
            h1b = bias_cols("head1_b", KI_IN, "h1b")
            gT = linear_ft(h1_t, h1b, out0, 1, "gT", alloc=1)

            # ---------------- phase 6: final head on image rows
            hw_t = load_w_tiles("head_w", KI_IN, KI_IN, "hww")
            hb = bias_cols("head_b", KI_IN, "hb")
            own0 = OWN[0]
            for c0 in range(0, own0, 512):
                w = min(512, own0 - c0)
                _, gf = rhs_wide(KI_IN, "gf")
                for ki in range(KI_IN):
                    load_feat(gf[ki], ki, c0, w)
                    nc.vector.tensor_scalar_add(
                        gf[ki][:, :w], gf[ki][:, :w], gT[ki][:, :1])
                oT = linear_ft(hw_t, hb, gf, w, "oT")
                for s0 in range(0, w, P):
                    sw = min(P, w - s0)
                    orow = act.tile([P, D_IN], BF16, tag="obf")
                    for mo in range(KI_IN):
                        pt = ps.tile([P, P], F32, tag="tp")
                        nc.tensor.transpose(
                            out=pt[:sw, :], in_=oT[mo][:, s0:s0 + sw],
                            identity=ident[:])
                        nc.vector.tensor_copy(
                            out=orow[:sw, mo * P:(mo + 1) * P],
                            in_=pt[:sw, :])
                    nc.sync.dma_start(
                        out=out[c0 + s0:c0 + s0 + sw, :], in_=orow[:sw, :])

    nc.compile()
    return nc


# ---------------------------------------------------------------- entry point

_CACHE = {}


def _get_compiled(inputs, cfg):
    in_maps, struct = prep(inputs, cfg)
    key = tuple(sorted((k, v["B"], v["C"]) for k, v in struct.items()
                       if k != "pack")) + (
        struct["pack"]["NW"], struct["pack"]["N16"], struct["pack"]["NF"],
        BUILD_MODE)
    if key not in _CACHE:
        _CACHE[key] = build(struct, cfg)
    return _CACHE[key], in_maps


def kernel(**inputs) -> np.ndarray:
    cfg = Cfg()
    nc, in_maps = _get_compiled(inputs, cfg)
    res = run_bass_kernel_spmd(nc, in_maps, list(range(NCORES)))
    return np.concatenate(
        [np.asarray(res.results[c]["out"]).astype(np.float32)
         for c in range(NCORES)], axis=0)


# revision 32
# speedup vs baseline: 1.7306x; 1.7306x over previous
"""HEATNet4 Bass/Tile kernel for 8 TRN2 NeuronCores.

Self-contained: takes FULL unsharded inputs, shards internally
(nodes row-sharded; edges sharded by dst owner), runs one SPMD Bass
program on cores 0-7, gathers the full [NI, 512] output.

I/O packing: the per-call dispatch overhead of this runtime scales
with the NUMBER of input tensors (~1.8 ms each) plus bytes, so all
inputs are packed into 4 tensors per core:
  fblob  bf16 [512, 4375]   transposed node features (img|gene|text)
  wshard f32  [NW/8]        1/8 shard of all weights (AllGathered on
                            device into wfull at kernel start)
  eb16   i16  [N16]         edge index data (16-row wrapped gather
                            indices, replicated to 128 on device)
  ebf    f32  [NF]          per-edge float data (dst pos, sim)
Runtime scalars (e_w/e_b folds, sigmoid(skip) blends) travel in
wshard as 128-replicated words read as [P,1] columns.
"""
import sys
sys.path.insert(0, "/opt/trn_rl_repo")

import math

import numpy as np

import concourse.bass as bass
import concourse.bacc as bacc
import concourse.mybir as mybir
import concourse.tile as tile
import concourse.bass_isa as bass_isa
from concourse.bass_utils import run_bass_kernel_spmd

F32 = mybir.dt.float32
BF16 = mybir.dt.bfloat16
I32 = mybir.dt.int32
I16 = mybir.dt.int16
F8 = mybir.dt.float8e4
U8 = mybir.dt.uint8
NPBF16 = mybir.dt.np(BF16)
NPF8 = mybir.dt.np(F8)
NCORES = 8
P = 128


class Cfg:
    NI, NG, NT = 20000, 10000, 5000
    D_IN, D, L, H = 512, 256, 2, 8
    DK = D // H
    E = 100000
    NS = [NI, NG, NT]
    OWN = [n // NCORES for n in NS]
    OWN_ALL = sum(OWN)
    TBASE = [0, OWN[0], OWN[0] + OWN[1]]


ETYPES = [  # (st, dt, suffix)
    (0, 1, "ig"), (1, 0, "gi"), (0, 2, "it"),
    (2, 0, "ti"), (1, 2, "gt"), (2, 1, "tg"),
]


# ---------------------------------------------------------------- host prep

def _wrap16(idx: np.ndarray) -> np.ndarray:
    """[n] -> [16, n/16] wrapped int16 layout for dma_gather idx."""
    n = idx.shape[0]
    return np.ascontiguousarray(idx.astype(np.int16).reshape(n // 16, 16).T)


def _bin_pack(deg: np.ndarray, B: int):
    """LPT: assign nodes to B bins (<=128 nodes each), balancing edge load."""
    import heapq
    order = np.argsort(-deg, kind="stable")
    bins = [[] for _ in range(B)]
    heap = [(0, b) for b in range(B)]
    heapq.heapify(heap)
    for n in order:
        while True:
            load, b = heapq.heappop(heap)
            if len(bins[b]) < P:
                break
        bins[b].append(int(n))
        heapq.heappush(heap, (load + int(deg[n]), b))
    return bins


class _Pk:
    """Packs named f32 arrays into one flat buffer, recording offsets."""

    def __init__(self):
        self.parts, self.offs, self.off = [], {}, 0

    def add(self, name, arr):
        a = np.ascontiguousarray(np.asarray(arr), ).astype(np.float32).ravel()
        self.offs[name] = self.off
        self.parts.append(a)
        self.off += a.size

    def finish(self, pad_mult):
        tot = self.off
        padded = ((tot + pad_mult - 1) // pad_mult) * pad_mult
        if padded > tot:
            self.parts.append(np.zeros(padded - tot, np.float32))
        return np.concatenate(self.parts), padded


def prep(inputs: dict, cfg: Cfg):
    """Host-side preprocessing. Returns (in_maps, struct)."""
    OWN, TBASE = cfg.OWN, cfg.TBASE
    e_w, e_b = np.asarray(inputs["e_w"]), np.asarray(inputs["e_b"])
    inv_sqrt_dk = 1.0 / math.sqrt(cfg.DK)

    struct = {}
    per_core_16 = [[] for _ in range(NCORES)]   # list of i16 flats
    o16 = 0

    for st, dt, sfx in ETYPES:
        src = np.asarray(inputs[f"src_{sfx}"]).astype(np.int64)
        dst = np.asarray(inputs[f"dst_{sfx}"]).astype(np.int64)
        sim = np.asarray(inputs[f"sim_{sfx}"]).astype(np.float32)
        own = OWN[dt]
        owner = dst // own

        cores = []
        for c in range(NCORES):
            eids = np.nonzero(owner == c)[0]
            dl = dst[eids] - c * own
            order = np.argsort(dl, kind="stable")
            eids = eids[order]
            dl = dl[order]
            deg = np.bincount(dl, minlength=own)
            starts = np.zeros(own + 1, np.int64)
            np.cumsum(deg, out=starts[1:])
            cores.append((eids, deg, starts))

        B0 = (own + P - 1) // P
        best = None
        for B in range(B0, B0 + 4):
            allbins = []
            C = 1
            for c in range(NCORES):
                bins = _bin_pack(cores[c][1], B)
                allbins.append(bins)
                for bn in bins:
                    load = int(cores[c][1][bn].sum()) if bn else 0
                    C = max(C, (load + P - 1) // P)
            tot = B * C
            if best is None or tot < best[0]:
                best = (tot, B, C, allbins)
        _, B, C, allbins = best
        nch = B * C
        epad = nch * P

        assert nch % 2 == 0
        struct[sfx] = dict(B=B, C=C, nch=nch,
                           o_srcg=o16, o_qg=o16 + 16 * nch * 8,
                           o_sidx=o16 + 32 * nch * 8,
                           o_dcol8=o16 + 32 * nch * 8 + P * B,
                           o_sim16=o16 + 32 * nch * 8 + P * B + P * nch // 2)
        o16 += 32 * nch * 8 + P * B + P * nch // 2 + P * nch

        for c in range(NCORES):
            eids, deg, starts = cores[c]
            src_arr = np.zeros(epad, np.int64)
            q_arr = np.zeros(epad, np.int64)
            pos_arr = np.full(epad, -1.0, np.float32)
            sim_arr = np.zeros(epad, np.float32)
            sidx = np.full((P, B), own, np.int64)
            for b, bn in enumerate(allbins[c]):
                cur = b * C * P
                for pos, n in enumerate(bn):
                    sidx[pos, b] = n
                    s0, s1 = starts[n], starts[n + 1]
                    k = s1 - s0
                    if k:
                        sel = eids[s0:s1]
                        src_arr[cur:cur + k] = src[sel]
                        q_arr[cur:cur + k] = TBASE[dt] + (dst[sel] - c * own)
                        pos_arr[cur:cur + k] = pos
                        sim_arr[cur:cur + k] = sim[sel]
                        cur += k
                assert cur <= (b + 1) * C * P

            dcol8 = np.where(pos_arr < 0, 255.0, pos_arr).astype(np.uint8)
            dcol8 = np.ascontiguousarray(dcol8.reshape(nch, P).T)
            sim16 = np.ascontiguousarray(
                sim_arr.reshape(nch, P).T.astype(NPBF16))
            per_core_16[c] += [_wrap16(src_arr).ravel(),
                               _wrap16(q_arr).ravel(),
                               sidx.astype(np.int16).ravel(),
                               dcol8.ravel().view(np.int16),
                               sim16.ravel().view(np.int16)]

    # ---- weights blob (shared across cores; shipped sharded):
    # f32 part (biases, runtime scalars, iota/ident) followed by a bf16
    # section holding all weight matrices (cast back to f32 on device)
    pk = _Pk()
    pk.add("adapt_b", inputs["adapt_b"])
    pk.add("k_b", inputs["k_b"])
    pk.add("q_b", inputs["q_b"])
    pk.add("v_b", inputs["v_b"])
    pk.add("a_b", inputs["a_b"])
    pk.add("pred_b", np.asarray(inputs["pred_b"], np.float32)[0])
    pk.add("head1_b", inputs["head1_b"])
    pk.add("head_b", inputs["head_b"])
    s_ew = (e_w * inv_sqrt_dk).astype(np.float32)      # [L]
    s_eb = (e_b * inv_sqrt_dk).astype(np.float32)
    pk.add("s_ew", np.repeat(s_ew, P))
    pk.add("s_eb", np.repeat(s_eb, P))
    alpha = 1.0 / (1.0 + np.exp(-np.asarray(inputs["skip"], np.float64)))
    pk.add("alpha", np.repeat(alpha.astype(np.float32).ravel(), P))
    pk.add("oma", np.repeat((1.0 - alpha).astype(np.float32).ravel(), P))
    pk.add("iota", np.tile(np.arange(P, dtype=np.float32), (P, 1)))
    pk.add("ident", np.eye(P, dtype=np.float32))
    wpart, WMAT = pk.finish(pad_mult=2)
    offs16, m16, o16w = {}, [], 0
    for name, arr in (
            ("adapt_w", inputs["adapt_w"]),
            ("k_w", inputs["k_w"]), ("q_w", inputs["q_w"]),
            ("v_w", inputs["v_w"]),
            ("a_w", np.asarray(inputs["a_w"], np.float32) * 0.5),
            ("pred_w", np.asarray(inputs["pred_w"], np.float32)[0] / cfg.NI),
            ("head1_w", inputs["head1_w"]),
            ("head_w", inputs["head_w"])):
        a = np.ascontiguousarray(np.asarray(arr)).astype(NPBF16).ravel()
        offs16[name] = o16w
        m16.append(a)
        o16w += a.size
    if o16w % 2:
        m16.append(np.zeros(1, NPBF16)); o16w += 1
    wblob = np.concatenate(
        [wpart.view(np.uint8), np.concatenate(m16).view(np.uint8)]
    ).view(np.float32)
    NW = WMAT + o16w // 2
    NW = ((NW + NCORES * 4 - 1) // (NCORES * 4)) * (NCORES * 4)
    SW = NW // NCORES
    wblob = np.concatenate([wblob, np.zeros(NW - wblob.size, np.float32)])

    # single-blob layout (f32 words):
    # [wshard | f_img(bf16) | f_gene+f_text(fp8) | eb16(i16, incl u8 dcol
    #  and bf16 sim bit-packed)]
    ICOLS = cfg.OWN[0]
    GTCOLS = cfg.OWN[1] + cfg.OWN[2]
    GTCOLS_P = GTCOLS + (-GTCOLS) % 4
    N16 = o16
    assert N16 % 2 == 0
    O_F = SW
    O_F8 = O_F + cfg.D_IN * ICOLS // 2
    O_16 = O_F8 + cfg.D_IN * GTCOLS_P // 4
    NB = O_16 + N16 // 2
    NB += (-NB) % 128
    struct["pack"] = dict(offs=pk.offs, offs16=offs16, WMAT=WMAT,
                          NW=NW, SW=SW, N16=N16,
                          ICOLS=ICOLS, GTCOLS_P=GTCOLS_P,
                          O_F=O_F, O_F8=O_F8, O_16=O_16, NB=NB)

    feats = [np.asarray(inputs["feat_image"], np.float32),
             np.asarray(inputs["feat_gene"], np.float32),
             np.asarray(inputs["feat_text"], np.float32)]
    in_maps = []
    for c in range(NCORES):
        fb = np.ascontiguousarray(
            feats[0][c * cfg.OWN[0]:(c + 1) * cfg.OWN[0]].T.astype(NPBF16))
        f8 = np.zeros((cfg.D_IN, GTCOLS_P), NPF8)
        f8[:, :GTCOLS] = np.concatenate(
            [feats[t][c * cfg.OWN[t]:(c + 1) * cfg.OWN[t]].T
             for t in (1, 2)], axis=1).astype(NPF8)
        blob = np.concatenate([
            wblob[c * SW:(c + 1) * SW].view(np.uint8),
            fb.view(np.uint8).ravel(),
            np.ascontiguousarray(f8).view(np.uint8).ravel(),
            np.concatenate(per_core_16[c]).view(np.uint8),
        ]).view(np.float32)
        blob = np.concatenate([blob, np.zeros(NB - blob.size, np.float32)])
        in_maps.append(dict(blob=blob.reshape(NB // 128, 128)))
    return in_maps, struct


# ---------------------------------------------------------------- device build

BUILD_MODE = "full"  # "full" | "nogather" (timing expt) | "stub" (dispatch baseline)


def build(struct, cfg: Cfg):
    OWN, TBASE, NS = cfg.OWN, cfg.TBASE, cfg.NS
    D, L, H, DK, D_IN = cfg.D, cfg.L, cfg.H, cfg.DK, cfg.D_IN
    OWN_ALL = cfg.OWN_ALL
    KI_IN, MO = D_IN // P, D // P  # 4, 2
    KI = D // P                    # 2
    PACK = struct["pack"]
    OFFS, NW, SW = PACK["offs"], PACK["NW"], PACK["SW"]

    nc = bacc.Bacc("TRN2", target_bir_lowering=False, debug=False,
                   num_devices=NCORES)

    # NOTE: declared 2-D — the transfer layer ships large 1-D tensors ~2x
    # slower than the same bytes declared [N/128, 128].
    blob2 = nc.dram_tensor("blob", [PACK["NB"] // 128, 128], F32,
                           kind="ExternalInput")
    blob = blob2.rearrange("r c -> (r c)")
    ICOLS, GTCOLS_P = PACK["ICOLS"], PACK["GTCOLS_P"]
    O_F, O_F8, O_16 = PACK["O_F"], PACK["O_F8"], PACK["O_16"]
    wshard = blob[0:SW]
    fblob = blob[O_F:O_F + D_IN * ICOLS // 2].bitcast(BF16).rearrange(
        "(r c) -> r c", c=ICOLS)
    f8blob = blob[O_F8:O_F8 + D_IN * GTCOLS_P // 4].bitcast(F8).rearrange(
        "(r c) -> r c", c=GTCOLS_P)

    def eb16v(o, sz, cols, dt=I16):
        assert o % 2 == 0 and sz % 2 == 0
        w = blob[O_16 + o // 2:O_16 + (o + sz) // 2]
        return w.bitcast(dt).rearrange("(r c) -> r c", c=cols)

    out = nc.dram_tensor("out", [OWN[0], D_IN], BF16, kind="ExternalOutput")

    # ---- internal DRAM
    wsh_int = nc.dram_tensor("wsh_int", [SW], F32)
    wfull = nc.dram_tensor("wfull", [NW], F32, addr_space="Shared")
    h_cur = [nc.dram_tensor(f"hA_{t}", [D, OWN[t]], F32) for t in range(3)]
    h_nxt = [nc.dram_tensor(f"hB_{t}", [D, OWN[t]], F32) for t in range(3)]
    kv_own = [nc.dram_tensor(f"kvown_{t}", [OWN[t], 2 * D], BF16)
              for t in range(3)]
    kv_full = [nc.dram_tensor(f"kvfull_{t}", [NS[t], 2 * D], BF16,
                              addr_space="Shared") for t in range(3)]
    q_loc = nc.dram_tensor("q_loc", [OWN_ALL, D], BF16)
    agg_t = {sfx: nc.dram_tensor(f"agg_{sfx}", [OWN[dt_] + 1, D], F32)
             for st, dt_, sfx in ETYPES}
    pool_in = nc.dram_tensor("pool_in", [D, 1], F32)
    pool_ar = nc.dram_tensor("pool_ar", [D, 1], F32, addr_space="Shared")

    RG = [list(range(NCORES))]

    def wv(name, rows, cols, extra_off=0):
        """2-D view of an f32 entry stored row-major in wfull."""
        o = OFFS[name] + extra_off
        return wfull[o:o + rows * cols].rearrange("(r c) -> r c", c=cols)

    def wv16(name, rows, cols, extra_off=0):
        """2-D view of a bf16 weight matrix in wfull's bf16 section."""
        o = PACK["offs16"][name] + extra_off
        base = PACK["WMAT"]
        return wfull[base + o // 2:base + (o + rows * cols) // 2].bitcast(
            BF16).rearrange("(r c) -> r c", c=cols)

    with tile.TileContext(nc) as tc:
        with (
            tc.tile_pool(name="cst", bufs=1) as cst,
            tc.tile_pool(name="wts", bufs=1) as wts,
            tc.tile_pool(name="act", bufs=2) as act,
            tc.tile_pool(name="gath", bufs=2) as gath,
            tc.tile_pool(name="etc", bufs=2) as etc_p,
            tc.tile_pool(name="sml", bufs=4) as sml,
            tc.tile_pool(name="ps", bufs=2, space="PSUM") as ps,
        ):
            # ---- gather full weight blob
            nc.sync.dma_start(out=wsh_int[:], in_=wshard[:])
            nc.gpsimd.collective_compute(
                "AllGather", mybir.AluOpType.bypass, replica_groups=RG,
                ins=[wsh_int[:]], outs=[wfull[:]])

            iota = cst.tile([P, P], F32)
            nc.sync.dma_start(out=iota[:], in_=wv("iota", P, P))
            ident = cst.tile([P, P], F32)
            nc.sync.dma_start(out=ident[:], in_=wv("ident", P, P))
            iota_w = cst.tile([P, 9 * P], F32, tag="iota_w")
            for g in range(9):
                nc.sync.dma_start(out=iota_w[:, g * P:(g + 1) * P],
                                  in_=wv("iota", P, P))
            # runtime scalar columns: [P, 1] views
            scols = cst.tile([P, 2 * L + 4 * L * 3], F32, tag="scols")
            nc.sync.dma_start(
                out=scols[:, 0:2 * L],
                in_=wfull[OFFS["s_ew"]:OFFS["s_ew"] + 2 * L * P].rearrange(
                    "(m p) -> p m", p=P))
            nc.sync.dma_start(
                out=scols[:, 2 * L:2 * L + 2 * L * 3],
                in_=wfull[OFFS["alpha"]:OFFS["alpha"] + 2 * L * 3 * P].rearrange(
                    "(m p) -> p m", p=P))
            s_ew_c = [scols[:, l:l + 1] for l in range(L)]
            s_eb_c = [scols[:, L + l:L + l + 1] for l in range(L)]
            al_c = [[scols[:, 2 * L + l * 3 + t:2 * L + l * 3 + t + 1]
                     for t in range(3)] for l in range(L)]
            oma_c = [[scols[:, 5 * L + l * 3 + t:5 * L + l * 3 + t + 1]
                      for t in range(3)] for l in range(L)]

            def load_w_tiles(name, n_ki, n_mo, tag, extra_off=0):
                w_ap = wv16(name, n_ki * P, n_mo * P, extra_off)
                wide16 = wts.tile([P, n_ki * n_mo * P], BF16, tag=tag + "16")
                for ki in range(n_ki):
                    for mo in range(n_mo):
                        j = (ki * n_mo + mo) * P
                        nc.sync.dma_start(
                            out=wide16[:, j:j + P],
                            in_=w_ap[ki * P:(ki + 1) * P, mo * P:(mo + 1) * P])
                wide = wts.tile([P, n_ki * n_mo * P], F32, tag=tag)
                nc.vector.tensor_copy(out=wide[:], in_=wide16[:])
                return [[wide[:, (ki * n_mo + mo) * P:(ki * n_mo + mo + 1) * P]
                         for mo in range(n_mo)] for ki in range(n_ki)]

            def bias_cols(name, n_mo, tag, extra_off=0):
                o = OFFS[name] + extra_off
                wide = sml.tile([P, n_mo], F32, tag=tag)
                nc.sync.dma_start(
                    out=wide[:],
                    in_=wfull[o:o + n_mo * P].rearrange("(m p) -> p m", p=P))
                return [wide[:, mo:mo + 1] for mo in range(n_mo)]

            def rhs_wide(n_ki, tag, pool=None):
                wide = (pool or act).tile([P, n_ki * 512], F32, tag=tag)
                return wide, [wide[:, ki * 512:(ki + 1) * 512]
                              for ki in range(n_ki)]

            def load_feat(dest_ap, ki, t, c0, w):
                """DMA feature slice (img: bf16, gene/text: fp8) + cast to
                f32 into dest_ap[:, :w]."""
                if t == 0:
                    bfst = gath.tile([P, 512], BF16, tag="bfst")
                    nc.sync.dma_start(
                        out=bfst[:, :w],
                        in_=fblob[ki * P:(ki + 1) * P, c0:c0 + w])
                    nc.vector.tensor_copy(out=dest_ap[:, :w], in_=bfst[:, :w])
                else:
                    col0 = TBASE[t] - OWN[0] + c0
                    f8st = gath.tile([P, 512], F8, tag="f8st")
                    nc.sync.dma_start(
                        out=f8st[:, :w],
                        in_=f8blob[ki * P:(ki + 1) * P, col0:col0 + w])
                    nc.vector.tensor_copy(out=dest_ap[:, :w], in_=f8st[:, :w])

            def linear_ft(w_tiles, b_cols, rhs_tiles, w, out_tag, alloc=512):
                n_ki = len(w_tiles)
                n_mo = len(w_tiles[0])
                ow = act.tile([P, n_mo * alloc], F32, tag=out_tag)
                outs = []
                for mo in range(n_mo):
                    psum = ps.tile([P, w], F32, tag="lin")
                    for ki in range(n_ki):
                        nc.tensor.matmul(out=psum[:], lhsT=w_tiles[ki][mo],
                                         rhs=rhs_tiles[ki][:, :w],
                                         start=(ki == 0), stop=(ki == n_ki - 1))
                    o = ow[:, mo * alloc:mo * alloc + w]
                    nc.vector.tensor_scalar_add(o, psum[:], b_cols[mo])
                    outs.append(ow[:, mo * alloc:(mo + 1) * alloc])
                return outs

            # ---------------- phase 0: adapt
            for t in range(3):
                w_tiles = load_w_tiles("adapt_w", KI_IN, MO, "adw",
                                       extra_off=t * D_IN * D)
                b_cols = bias_cols("adapt_b", MO, "adb", extra_off=t * D)
                own = OWN[t]
                for c0 in range(0, own, 512):
                    w = min(512, own - c0)
                    _, rhs = rhs_wide(KI_IN, "gf")
                    for ki in range(KI_IN):
                        load_feat(rhs[ki], ki, t, c0, w)
                    houts = linear_ft(w_tiles, b_cols, rhs, w, "hout")
                    for mo in range(MO):
                        nc.sync.dma_start(
                            out=h_cur[t][mo * P:(mo + 1) * P, c0:c0 + w],
                            in_=houts[mo][:, :w])

            # ---------------- layers
            for l in range(L if BUILD_MODE != "stub" else 0):
                # phase 1: k/q/v linears + transpose to row tables
                for t in range(3):
                    own = OWN[t]
                    eo = (l * 3 + t) * D * D
                    eob = (l * 3 + t) * D
                    kw_t = load_w_tiles("k_w", KI, MO, "kw", eo)
                    qw_t = load_w_tiles("q_w", KI, MO, "qw", eo)
                    vw_t = load_w_tiles("v_w", KI, MO, "vw", eo)
                    kb = bias_cols("k_b", MO, "kb", eob)
                    qb = bias_cols("q_b", MO, "qb", eob)
                    vb = bias_cols("v_b", MO, "vb", eob)
                    for c0 in range(0, own, 512):
                        w = min(512, own - c0)
                        _, rhs = rhs_wide(KI, "hrhs")
                        for ki in range(KI):
                            nc.sync.dma_start(
                                out=rhs[ki][:, :w],
                                in_=h_cur[t][ki * P:(ki + 1) * P, c0:c0 + w])
                        kT = linear_ft(kw_t, kb, rhs, w, "kT")
                        qT = linear_ft(qw_t, qb, rhs, w, "qT")
                        vT = linear_ft(vw_t, vb, rhs, w, "vT")
                        for s0 in range(0, w, P):
                            sw = min(P, w - s0)
                            kvrow = act.tile([P, 2 * D], BF16, tag="kvrow")
                            qrow = act.tile([P, D], BF16, tag="qrow")
                            for mo in range(MO):
                                for src_t, dst_col, buf in (
                                        (kT[mo], mo * P, kvrow),
                                        (vT[mo], D + mo * P, kvrow),
                                        (qT[mo], mo * P, qrow)):
                                    pt = ps.tile([P, P], F32, tag="tp")
                                    nc.tensor.transpose(
                                        out=pt[:sw, :],
                                        in_=src_t[:, s0:s0 + sw],
                                        identity=ident[:])
                                    nc.vector.tensor_copy(
                                        out=buf[:sw, dst_col:dst_col + P],
                                        in_=pt[:sw, :])
                            r0 = c0 + s0
                            nc.sync.dma_start(
                                out=kv_own[t][r0:r0 + sw, :],
                                in_=kvrow[:sw, :])
                            nc.sync.dma_start(
                                out=q_loc[TBASE[t] + r0:TBASE[t] + r0 + sw, :],
                                in_=qrow[:sw, :])
                    # allgather this type's kv table right away so the
                    # collective overlaps the next type's phase-1 compute
                    if BUILD_MODE == "nocoll":
                        nc.sync.dma_start(out=kv_full[t][0:OWN[t], :],
                                          in_=kv_own[t][:])
                    else:
                        nc.gpsimd.collective_compute(
                            "AllGather", mybir.AluOpType.bypass,
                            replica_groups=RG,
                            ins=[kv_own[t][:]], outs=[kv_full[t][:]])

                # phase 3: message passing per etype
                for st, dt_, sfx in ETYPES:
                    S_ = struct[sfx]; B, C, nch = S_["B"], S_["C"], S_["nch"]
                    n16 = nch * 8
                    srcg = etc_p.tile([P, n16], I16, tag="srcg")
                    qgi = etc_p.tile([P, n16], I16, tag="qgi")
                    for name, tl in (("o_srcg", srcg), ("o_qg", qgi)):
                        v16 = eb16v(S_[name], 16 * n16, n16)
                        for k in range(8):
                            nc.sync.dma_start(
                                out=tl[16 * k:16 * (k + 1), :], in_=v16)
                    dcol8 = etc_p.tile([P, nch], U8, tag="dcol8")
                    nc.sync.dma_start(
                        out=dcol8[:],
                        in_=eb16v(S_["o_dcol8"], P * nch // 2, nch, dt=U8))
                    dcol = etc_p.tile([P, nch], F32, tag="dcol")
                    nc.vector.tensor_copy(out=dcol[:], in_=dcol8[:])
                    simt16 = etc_p.tile([P, nch], BF16, tag="simt16")
                    nc.sync.dma_start(
                        out=simt16[:],
                        in_=eb16v(S_["o_sim16"], P * nch, nch, dt=BF16))
                    simt = etc_p.tile([P, nch], F32, tag="simt")
                    nc.vector.tensor_copy(out=simt[:], in_=simt16[:])
                    ea = etc_p.tile([P, nch], F32, tag="ea")
                    nc.vector.tensor_scalar(
                        out=ea[:], in0=simt[:], scalar1=s_ew_c[l],
                        scalar2=s_eb_c[l], op0=mybir.AluOpType.mult,
                        op1=mybir.AluOpType.add)
                    sidx16 = etc_p.tile([P, B], I16, tag="sidx16")
                    nc.sync.dma_start(
                        out=sidx16[:], in_=eb16v(S_["o_sidx"], P * B, B))
                    sidx = etc_p.tile([P, B], I32, tag="sidx")
                    nc.vector.tensor_copy(out=sidx[:], in_=sidx16[:])

                    # single-pass message passing: gather full kv
                    # rows (k|v contiguous -> one gpsimd call) + q rows in
                    # waves; batch score/mask/weight math per wave; two PE
                    # matmuls per chunk accumulate numerator+denominator per
                    # dst block. Scores are O(5) so exp needs no max-shift.
                    GW = 9        # chunks per gather wave (desc ring <=2048)
                    psum = None
                    for k0 in range(0, nch, GW):
                        gc = min(GW, nch - k0)
                        kvg = gath.tile([P, GW, 2 * D], BF16, tag="kvg")
                        qg = gath.tile([P, GW, D], BF16, tag="qg")
                        if BUILD_MODE == "nogather":
                            nc.sync.dma_start(
                                out=kvg[:, :gc, :],
                                in_=kv_full[st][0:gc * P, :].rearrange(
                                    "(g p) d -> p g d", p=P))
                            nc.sync.dma_start(
                                out=qg[:, :gc, :],
                                in_=q_loc[0:gc * P, :].rearrange(
                                    "(g p) d -> p g d", p=P))
                        else:
                            nc.gpsimd.dma_gather(
                                kvg[:, :gc, :], kv_full[st][:],
                                srcg[:, k0 * 8:(k0 + gc) * 8],
                                gc * P, gc * P, 2 * D)
                            nc.gpsimd.dma_gather(
                                qg[:, :gc, :], q_loc[:],
                                qgi[:, k0 * 8:(k0 + gc) * 8],
                                gc * P, gc * P, D)
                        prod = gath.tile([P, GW, D], F32, tag="prod")
                        nc.vector.tensor_mul(
                            prod[:, :gc, :], kvg[:, :gc, 0:D], qg[:, :gc, :])
                        scw = gath.tile([P, GW, H], F32, tag="scw")
                        nc.vector.tensor_reduce(
                            out=scw[:, :gc, :],
                            in_=prod[:, :gc, :].rearrange(
                                "p g (h k) -> p g h k", k=DK),
                            axis=mybir.AxisListType.X, op=mybir.AluOpType.add)
                        nc.vector.tensor_mul(
                            scw[:, :gc, :], scw[:, :gc, :],
                            ea[:, k0:k0 + gc].to_broadcast([P, gc, H]))
                        nc.scalar.activation(
                            out=scw[:, :gc, :].rearrange("p g h -> p (g h)"),
                            in_=scw[:, :gc, :].rearrange("p g h -> p (g h)"),
                            func=mybir.ActivationFunctionType.Exp)
                        smw = gath.tile([P, GW, P], F32, tag="smw")
                        nc.vector.tensor_tensor(
                            out=smw[:, :gc, :],
                            in0=dcol[:, k0:k0 + gc].to_broadcast([P, gc, P]),
                            in1=iota_w[:, 0:gc * P].rearrange(
                                "p (g x) -> p g x", x=P),
                            op=mybir.AluOpType.is_equal)
                        vprod = gath.tile([P, GW, D], F32, tag="vprod")
                        nc.vector.tensor_mul(
                            vprod[:, :gc, :].rearrange(
                                "p g (h k) -> p g h k", k=DK),
                            kvg[:, :gc, D:2 * D].rearrange(
                                "p g (h k) -> p g h k", k=DK),
                            scw[:, :gc, :].to_broadcast([P, gc, H, DK]))
                        for j in range(gc):
                            k = k0 + j
                            ci = k % C
                            b = k // C
                            if ci == 0:
                                psum = ps.tile([P, H + D], F32, tag="blk")
                            rhs = etc_p.tile([P, H + D], F32, tag="rhs")
                            nc.vector.tensor_copy(out=rhs[:, 0:H],
                                                  in_=scw[:, j, :])
                            nc.vector.tensor_copy(out=rhs[:, H:H + D],
                                                  in_=vprod[:, j, :])
                            nc.tensor.matmul(
                                out=psum[:], lhsT=smw[:, j, :], rhs=rhs[:],
                                start=(ci == 0), stop=(ci == C - 1))
                            if ci == C - 1:
                                s_t = sml.tile([P, H], F32, tag="s_t")
                                nc.vector.tensor_scalar_max(
                                    s_t[:], psum[:, 0:H], 1e-30)
                                r_t = sml.tile([P, H], F32, tag="r_t")
                                nc.vector.reciprocal(r_t[:], s_t[:])
                                aggsc = etc_p.tile([P, D], F32, tag="aggsc")
                                nc.vector.tensor_mul(
                                    aggsc[:].rearrange(
                                        "p (h k) -> p h k", k=DK),
                                    psum[:, H:H + D].rearrange(
                                        "p (h k) -> p h k", k=DK),
                                    r_t[:].to_broadcast([P, H, DK]))
                                nc.gpsimd.indirect_dma_start(
                                    out=agg_t[sfx][:],
                                    out_offset=bass.IndirectOffsetOnAxis(
                                        ap=sidx[:, b:b + 1], axis=0),
                                    in_=aggsc[:], in_offset=None)

                # phase 4: combine aggs, a_w transform, blend
                for t in range(3):
                    own = OWN[t]
                    sfxs = [sfx for st, dt_, sfx in ETYPES if dt_ == t]
                    eo = (l * 3 + t) * D * D
                    eob = (l * 3 + t) * D
                    aw_t = load_w_tiles("a_w", KI, MO, "aw", eo)
                    ab = bias_cols("a_b", MO, "ab", eob)
                    for r0 in range(0, own, P):
                        rw = min(P, own - r0)
                        asum = act.tile([P, D], F32, tag="asum")
                        a2 = act.tile([P, D], F32, tag="a2")
                        nc.sync.dma_start(
                            out=asum[:rw, :],
                            in_=agg_t[sfxs[0]][r0:r0 + rw, :])
                        nc.sync.dma_start(
                            out=a2[:rw, :], in_=agg_t[sfxs[1]][r0:r0 + rw, :])
                        nc.vector.tensor_add(
                            asum[:rw, :], asum[:rw, :], a2[:rw, :])
                        aT_w = act.tile([P, KI * P], F32, tag="aT")
                        aT = []
                        for ki in range(KI):
                            pt = ps.tile([P, P], F32, tag="tp")
                            nc.tensor.transpose(
                                out=pt[:, :rw],
                                in_=asum[:rw, ki * P:(ki + 1) * P],
                                identity=ident[:rw, :rw])
                            a_sb = aT_w[:, ki * P:(ki + 1) * P]
                            nc.vector.tensor_copy(a_sb[:, :rw], pt[:, :rw])
                            aT.append(a_sb)
                        for mo in range(MO):
                            psum = ps.tile([P, P], F32, tag="lin")
                            for ki in range(KI):
                                nc.tensor.matmul(
                                    out=psum[:, :rw], lhsT=aw_t[ki][mo][:],
                                    rhs=aT[ki][:, :rw],
                                    start=(ki == 0), stop=(ki == KI - 1))
                            hold = act.tile([P, P], F32, tag="hold")
                            nc.sync.dma_start(
                                out=hold[:, :rw],
                                in_=h_cur[t][mo * P:(mo + 1) * P, r0:r0 + rw])
                            # new = (psum + ab) * al + hold * (1 - al)
                            tr = act.tile([P, P], F32, tag="tr")
                            nc.vector.tensor_scalar(
                                out=tr[:, :rw], in0=psum[:, :rw],
                                scalar1=ab[mo], scalar2=al_c[l][t],
                                op0=mybir.AluOpType.add,
                                op1=mybir.AluOpType.mult)
                            nc.vector.tensor_scalar_mul(
                                hold[:, :rw], hold[:, :rw], oma_c[l][t])
                            nc.vector.tensor_add(
                                tr[:, :rw], tr[:, :rw], hold[:, :rw])
                            nc.sync.dma_start(
                                out=h_nxt[t][mo * P:(mo + 1) * P, r0:r0 + rw],
                                in_=tr[:, :rw])
                h_cur, h_nxt = h_nxt, h_cur

            # ---------------- phase 5: pool image + head
            for ki in range(KI if BUILD_MODE != "stub" else 0):
                pcol = sml.tile([P, 1], F32, tag="pcol")
                psub = sml.tile([P, 1], F32, tag="psub")
                for i, c0 in enumerate(range(0, OWN[0], 512)):
                    w = min(512, OWN[0] - c0)
                    htile = act.tile([P, 512], F32, tag="hrhs")
                    nc.sync.dma_start(
                        out=htile[:, :w],
                        in_=h_cur[0][ki * P:(ki + 1) * P, c0:c0 + w])
                    tgt = pcol if i == 0 else psub
                    nc.vector.tensor_reduce(
                        out=tgt[:], in_=htile[:, :w],
                        axis=mybir.AxisListType.X, op=mybir.AluOpType.add)
                    if i > 0:
                        nc.vector.tensor_add(pcol[:], pcol[:], psub[:])
                nc.sync.dma_start(
                    out=pool_in[ki * P:(ki + 1) * P, :], in_=pcol[:])
            if BUILD_MODE == "nocoll":
                nc.sync.dma_start(out=pool_ar[:], in_=pool_in[:])
            elif BUILD_MODE != "stub":
                nc.gpsimd.collective_compute(
                    "AllReduce", mybir.AluOpType.add, replica_groups=RG,
                    ins=[pool_in[:]], outs=[pool_ar[:]])
            pooled_w = sml.tile([P, KI], F32, tag="pooled")
            pooled = []
            for ki in range(KI):
                if BUILD_MODE == "stub":
                    nc.gpsimd.memset(pooled_w[:, ki:ki + 1], 0.0)
                else:
                    nc.sync.dma_start(
                        out=pooled_w[:, ki:ki + 1],
                        in_=pool_ar[ki * P:(ki + 1) * P, :])
                pooled.append(pooled_w[:, ki:ki + 1])
            pw_t = load_w_tiles("pred_w", KI, MO, "pw")
            pb = bias_cols("pred_b", MO, "pb")
            out0 = linear_ft(pw_t, pb, pooled, 1, "out0", alloc=1)
            h1_t = load_w_tiles("head1_w", KI, KI_IN, "h1w")
            h1b = bias_cols("head1_b", KI_IN, "h1b")
            gT = linear_ft(h1_t, h1b, out0, 1, "gT", alloc=1)

            # ---------------- phase 6: final head on image rows
            hw_t = load_w_tiles("head_w", KI_IN, KI_IN, "hww")
            hb = bias_cols("head_b", KI_IN, "hb")
            own0 = OWN[0]
            for c0 in range(0, own0, 512):
                w = min(512, own0 - c0)
                _, gf = rhs_wide(KI_IN, "gf")
                for ki in range(KI_IN):
                    load_feat(gf[ki], ki, 0, c0, w)
                    nc.vector.tensor_scalar_add(
                        gf[ki][:, :w], gf[ki][:, :w], gT[ki][:, :1])
                oT = linear_ft(hw_t, hb, gf, w, "oT")
                for s0 in range(0, w, P):
                    sw = min(P, w - s0)
                    orow = act.tile([P, D_IN], BF16, tag="obf")
                    for mo in range(KI_IN):
                        pt = ps.tile([P, P], F32, tag="tp")
                        nc.tensor.transpose(
                            out=pt[:sw, :], in_=oT[mo][:, s0:s0 + sw],
                            identity=ident[:])
                        nc.vector.tensor_copy(
                            out=orow[:sw, mo * P:(mo + 1) * P],
                            in_=pt[:sw, :])
                    nc.sync.dma_start(
                        out=out[c0 + s0:c0 + s0 + sw, :], in_=orow[:sw, :])

    nc.compile()
    return nc


# ---------------------------------------------------------------- entry point

_CACHE = {}


def _get_compiled(inputs, cfg):
    in_maps, struct = prep(inputs, cfg)
    key = tuple(sorted((k, v["B"], v["C"]) for k, v in struct.items()
                       if k != "pack")) + (
        struct["pack"]["NW"], struct["pack"]["N16"], struct["pack"]["NF"],
        BUILD_MODE)
    if key not in _CACHE:
        _CACHE[key] = build(struct, cfg)
    return _CACHE[key], in_maps


def kernel(**inputs) -> np.ndarray:
    cfg = Cfg()
    nc, in_maps = _get_compiled(inputs, cfg)
    res = run_bass_kernel_spmd(nc, in_maps, list(range(NCORES)))
    return np.concatenate(
        [np.asarray(res.results[c]["out"]).astype(np.float32)
         for c in range(NCORES)], axis=0)
